# revision 2
# baseline (speedup 1.0000x reference)
# Banded (sliding-window) attention kernel for 8 TRN2 NeuronCores — v2.
#
# Problem: B=4, S=4096, HID=768, NH=12, D=64, one-sided window W=128.
# Sharding: core = b*2 + g (4 batches x 2 head-groups of 6 heads), fully
# independent cores, no collectives.
#
# v2 changes over v1 (213us):
#  - hT lives stripe-major ([128, stripe, ktile, 512]) so each projection
#    stripe arrives as ONE contiguous-6KB-line DMA.  Each DMA instruction
#    costs ~4.7us of queue service regardless of size, so ALL ramp-
#    critical inputs travel as exactly TWO bundle DMAs (sync: wq|wv|hT
#    stripe 0; scalar: wk|masks|hT stripe 1), host-concatenated; stripes
#    2-7 follow individually, alternating queues.
#    (fp8 was tried twice: full-fp8 DoubleRow projections are 2x faster
#    on PE but cost 2.1-3.7% norm rel vs the 2e-2 gate; a 1/3-fp8 hybrid
#    DR matmul appended to the bf16 chains measured 1.1e-2 error but ran
#    27us SLOWER -- the DR matmul stalls the bf16 LDW/MM pipeline.)
#  - Score matmuls for a head pair are emitted back-to-back with explicit
#    tile_position (0,0)/(64,0) so the two K=64 matmuls run concurrently
#    in separate 64-row groups of the PE array.
#  - ctx PSUM for all 6 heads of a chunk lives in one 3-bank tile
#    [65, 3pair, 4(sub0,sub1,pad,pad), 128]; one strided DVE copy
#    evacuates it (925ns vs 3x400) and one DMA per chunk ships it.
#  - Output is bf16 (half the output DMA bytes).
#
# Per-core pipeline (bf16 TensorE math, f32 PSUM accumulation):
#   for each 512-col projection stripe: project Q,K (d-major per pair)
#   and V (s-major, ones column for the softmax denominator), then run
#   the banded-attention key-tiles the stripe unblocks:
#     key-tile j: scores S_T[y, x] = K_j^T Q over query span (j-1..j+2)*128
#     (per-head PSUM bank), exp on ScalarE (band scale fused), triangular
#     0/1 band masks on VectorE, then PV with V stationary accumulating
#     ctx_T [65, 3, 4, 128] per chunk, one DVE evacuation, one DMA.
#   Normalization (divide by denominator) and the V-bias add happen on host.
#
# Output per core: [C=32, 65, 3, 2, 128] bf16 = (chunk, d|rowsum, pair, sub, x).

import numpy as np
import ml_dtypes

B, S, HID, NH, W = 4, 4096, 768, 12, 128
D = HID // NH          # 64
C = S // W             # 32 chunks / key-tiles
NHL = 6                # heads per core
NPAIR = 3              # head pairs per core (2 heads share 128 partitions)
KD = HID // 128        # 6 hidden k-tiles
BF16 = ml_dtypes.bfloat16

_CACHE = {}


def _build_nc():
    import concourse.bacc as bacc
    import concourse.tile as tile
    from concourse import mybir

    f32 = mybir.dt.float32
    bf16 = mybir.dt.bfloat16

    nc = bacc.Bacc(
        "TRN2", target_bir_lowering=False, debug=False, num_devices=8
    )

    # hT host-packed stripe-major [128, NSTRIPE, KD, 512]; weights
    # host-packed pair-major [128, NPAIR, KD, 128].
    WCOLS = NPAIR * KD * 128   # 2304 weight cols per tensor
    SBH = KD * 512             # 3072 cols per hT stripe
    hT_d = nc.dram_tensor("hT", [128, HID * S // 128], bf16, kind="ExternalInput")
    # ramp bundles: each fast queue gets ONE ~960KB DMA carrying a weight
    # tensor + half of stripe 0 (per-DMA queue service is ~4.7us + ~bytes/
    # 130GB/s, so fewer medium DMAs beat both many-small and one-huge)
    bun1_d = nc.dram_tensor("bun1", [128, WCOLS + SBH // 2], bf16,
                            kind="ExternalInput")
    bun2_d = nc.dram_tensor("bun2", [128, WCOLS + SBH // 2], bf16,
                            kind="ExternalInput")
    wv_d = nc.dram_tensor("wv", [128, WCOLS], bf16, kind="ExternalInput")
    mask_d = nc.dram_tensor("masks", [128, 4 * W], bf16, kind="ExternalInput")
    out_d = nc.dram_tensor("out", [C, D + 1, NPAIR, 2, W], bf16,
                           kind="ExternalOutput")

    NS = 512               # projection stripe (free dim)
    NSTRIPE = S // NS      # 8

    with tile.TileContext(nc) as tc:
        with (
            tc.tile_pool(name="persist", bufs=1) as persist,
            tc.tile_pool(name="probs", bufs=4) as probs_pool,
            tc.tile_pool(name="stage", bufs=3) as stage_pool,
            # PSUM: 8 banks. proj 2x1, score 3x1, ctx 1x3. Consecutive
            # matmuls must hit different banks (same-bank back-to-back
            # serializes on the ~166ns pipeline drain), so independent
            # chains are interleaved everywhere below.
            tc.tile_pool(name="proj_ps", bufs=2, space="PSUM") as proj_ps,
            tc.tile_pool(name="score_ps", bufs=3, space="PSUM") as score_ps,
            tc.tile_pool(name="ctx_ps", bufs=1, space="PSUM") as ctx_ps,
        ):
            # ---- persistent SBUF buffers ----
            hT = persist.tile([128, NSTRIPE, KD, NS], bf16, tag="hT")
            bun1 = persist.tile([128, WCOLS + SBH // 2], bf16, tag="bun1")
            bun2 = persist.tile([128, WCOLS + SBH // 2], bf16, tag="bun2")
            wv = persist.tile([128, NPAIR, KD, 128], bf16, tag="wv")
            masks = persist.tile([128, 2, 2, W], bf16, tag="masks")

            def wview(bun):
                return bun[:, 0:WCOLS].rearrange(
                    "p (a k c) -> p a k c", a=NPAIR, k=KD, c=128)

            wq = wview(bun1)
            wk = wview(bun2)
            hs0a = bun1[:, WCOLS:].rearrange("p (k c) -> p k c", k=3, c=NS)
            hs0b = bun2[:, WCOLS:].rearrange("p (k c) -> p k c", k=3, c=NS)

            def hTs(n, k):
                # [128, 512] slice of stripe n, k-tile k
                if n == 0:
                    return (hs0a, hs0b)[k // 3][:, k % 3, :]
                return hT[:, n, k, :]
            qdm = [persist.tile([128, S], bf16, tag=f"q{p}", name=f"q{p}")
                   for p in range(NPAIR)]
            kdm = [persist.tile([128, S], bf16, tag=f"k{p}", name=f"k{p}")
                   for p in range(NPAIR)]
            # V s-major with interleaved denominator column: [s-tile, head, 65]
            vsm = persist.tile([128, C, NHL, D + 1], bf16, tag="vsm")

            # ---- input DMAs.  Queue completion ~= 4.7us latency + bytes
            # at ~130GB/s, FIFO per queue: order strictly by needed-by time,
            # smallest-first, balanced across the two fast queues.
            PBW = KD * 128  # bundle cols per weight pair
            nc.sync.dma_start(bun1[:, 0:PBW], bun1_d[:, 0:PBW])
            nc.scalar.dma_start(bun2[:, 0:PBW], bun2_d[:, 0:PBW])
            nc.sync.dma_start(bun1[:, WCOLS:], bun1_d[:, WCOLS:])
            nc.scalar.dma_start(bun2[:, WCOLS:], bun2_d[:, WCOLS:])
            nc.sync.dma_start(bun1[:, PBW:WCOLS], bun1_d[:, PBW:WCOLS])
            nc.scalar.dma_start(bun2[:, PBW:WCOLS], bun2_d[:, PBW:WCOLS])
            nc.gpsimd.dma_start(wv[:], wv_d[:])
            nc.gpsimd.dma_start(masks[:], mask_d[:])
            HB = SBH // 2
            nc.sync.dma_start(hT[:, 1, 0:3], hT_d[:, SBH:SBH + HB])
            nc.scalar.dma_start(hT[:, 1, 3:6], hT_d[:, SBH + HB:2 * SBH])
            for n in range(2, NSTRIPE):
                eng = (nc.scalar, nc.sync)[n % 2]
                eng.dma_start(hT[:, n], hT_d[:, n * SBH:(n + 1) * SBH])
            # ones column for the PV denominator
            nc.vector.memset(vsm[:, :, :, D:D + 1], 1.0)

            # ---- HAM warm-up: ~6us of zero matmuls while the ramp DMAs
            # land, so the PE clock-gate is at 8/8 when real work starts.
            warm_sb = persist.tile([128, NS], bf16, tag="warm")
            nc.vector.memset(warm_sb[:], 0.0)
            wps = [proj_ps.tile([128, NS], f32, tag="proj", name="warm_ps")
                   for _ in range(2)]
            for i in range(14):
                nc.tensor.matmul(
                    wps[i % 2][:], warm_sb[:, 0:128], warm_sb[:],
                    start=True, stop=True,
                )

            # ---- fused projection + attention pipeline ----
            ptiles = [None] * C

            def emit_qk_proj_unit(n, p):
                # Q and K accumulation chains interleaved (alternating banks)
                psq = proj_ps.tile([128, NS], f32, tag="proj",
                                   name="proj_ps_q")
                psk = proj_ps.tile([128, NS], f32, tag="proj",
                                   name="proj_ps_k")
                for k in range(KD):
                    for ps, w in ((psq, wq), (psk, wk)):
                        nc.tensor.matmul(
                            ps[:],
                            w[:, p, k, :],
                            hTs(n, k),
                            start=(k == 0), stop=(k == KD - 1),
                        )
                nc.vector.tensor_copy(qdm[p][:, n * NS:(n + 1) * NS], psq[:])
                nc.vector.tensor_copy(kdm[p][:, n * NS:(n + 1) * NS], psk[:])

            def emit_v_proj_unit(sta):
                # two V s-tile chains interleaved
                psa = proj_ps.tile([128, NHL, D], f32, tag="proj",
                                   name="vproj_ps_a")
                psb = proj_ps.tile([128, NHL, D], f32, tag="proj",
                                   name="vproj_ps_b")
                for k in range(KD):
                    for ps, st in ((psa, sta), (psb, sta + 1)):
                        nc.tensor.matmul(
                            ps[:],
                            hTs(st // 4, k)[:, (st % 4) * 128:(st % 4 + 1) * 128],
                            wv[:, :, k, :],
                            start=(k == 0), stop=(k == KD - 1),
                        )
                nc.vector.tensor_copy(vsm[:, sta, :, 0:D], psa[:])
                nc.vector.tensor_copy(vsm[:, sta + 1, :, 0:D], psb[:])

            def proj_units(n, v_first=True):
                # As fillers, V units go first/early: their DVE evacuations
                # feed the next group's PV LDWEIGHTS.  In the prologue, ALL
                # QK units go first: pair-0 weights + stripe 0 are the only
                # ramp-critical DMAs; wv arrives while the QK chains run
                # (the PE queue is strict FIFO, so a stalled V unit would
                # block later QK units emitted behind it).
                qk = [lambda p=p: emit_qk_proj_unit(n, p) for p in range(NPAIR)]
                v = [lambda sta=sta: emit_v_proj_unit(sta)
                     for sta in (n * 4, n * 4 + 2)]
                if v_first:
                    return [v[0], qk[0], v[1], qk[1], qk[2]]
                return [qk[0], qk[1], qk[2], v[0], v[1]]

            def emit_step(j, c, fillers=()):
                # key-tile j scores (QK + exp + mask), interleaved with the
                # PV matmuls of chunk c = j-2.  P tile slice pi:
                # 0 -> chunk j-1 (mask x>=y), 1 -> chunk j, 2 -> chunk j+1
                # (mask x<=y).
                pv_mms = []
                if c is not None:
                    stage = stage_pool.tile([D + 1, NPAIR, 2, W],
                                            mybir.dt.bfloat16,
                                            tag="stage", name="stage_t")
                    ts = [t for t in (c - 1, c, c + 1) if 0 <= t < C]
                    if c == C - 1:
                        # the last chunk borrows 3 banks from the (by now
                        # idle) proj/score pools so its PV doesn't wait on
                        # the previous chunk's ctx evacuation
                        cps = [
                            proj_ps.tile([D + 1, 4, W], mybir.dt.float32,
                                         tag="proj", name="ctx_last_a"),
                            proj_ps.tile([D + 1, 4, W], mybir.dt.float32,
                                         tag="proj", name="ctx_last_b"),
                            score_ps.tile([D + 1, 4, W], mybir.dt.float32,
                                          tag="score", name="ctx_last_c"),
                        ]
                        sl = lambda p, sub: cps[p][:, sub, :]
                    else:
                        # one 3-bank ctx tile: [65, pair, sub(pad 4), 128]
                        cps = ctx_ps.tile([D + 1, NPAIR, 4, W],
                                          mybir.dt.float32,
                                          tag="ctx", name="ctx_ps_t")
                        sl = lambda p, sub: cps[:, p, sub, :]
                    # sub-outer: groups sharing a ctx bank stay sequential
                    # (start=True clears the whole bank's has_written bits);
                    # pair-inner: consecutive matmuls rotate across the 3
                    # ctx banks so they stream without drain serialization.
                    for sub in range(2):
                        for i, t in enumerate(ts):
                            for p in range(NPAIR):
                                pv_mms.append((
                                    sl(p, sub),
                                    vsm[:, t, p * 2 + sub, :],
                                    (t, p * 2 + sub, c - t + 1),
                                    i == 0, i == len(ts) - 1,
                                ))

                def drain_pv(k):
                    # no PV before h2: chunk c's ctx tile is WAR-blocked on
                    # chunk c-1's DVE evacuation (ctx pool rotation distance
                    # is one chunk); by h2 that has long retired.
                    if k < 2:
                        return
                    while pv_mms:
                        out, lhsT, (t, h, pi), st_, sp_ = pv_mms.pop(0)
                        nc.tensor.matmul(
                            out, lhsT, ptiles[t][:, h, pi, :],
                            start=st_, stop=sp_,
                        )

                if j is not None:
                    x0 = max(0, (j - 1) * 128)
                    x1 = min(S, (j + 2) * 128)
                    c0 = x0 - (j - 1) * 128
                    c1 = c0 + (x1 - x0)
                    s0, s1 = c0 // 128, (c1 - 1) // 128 + 1
                    pj = probs_pool.tile([128, NHL, 3, W], bf16, tag="P",
                                         name="P_t")
                    ptiles[j] = pj
                    for p in range(NPAIR):
                        if p == 2 and fillers:
                            # a ~1.3us projection chain absorbs the
                            # score-bank WAR wait (pair 2's banks free only
                            # after earlier exps retire)
                            fillers[0]()
                        pss = []
                        # the two K=64 score matmuls of a pair target
                        # disjoint 64-row groups -> back-to-back emission
                        # lets them run concurrently in the PE array
                        for sub in range(2):
                            bp = sub * 64
                            ps = score_ps.tile([128, 4 * W], f32, tag="score",
                                               name="score_ps_t")
                            pss.append(ps)
                            nc.tensor.matmul(
                                ps[:, c0:c1],
                                kdm[p][bp:bp + 64, j * 128:(j + 1) * 128],
                                qdm[p][bp:bp + 64, x0:x1],
                                start=True, stop=True,
                                tile_position=(bp, 0),
                            )
                        for sub in range(2):
                            h = p * 2 + sub
                            nc.scalar.activation(
                                pj[:, h, s0:s1, :], pss[sub][:, c0:c1],
                                mybir.ActivationFunctionType.Exp,
                                scale=1.0 / float(np.sqrt(D)),
                            )
                            drain_pv(p * 2 + sub)
                drain_pv(5)
                if c is not None:
                    # ctx evacuation FIRST on the DVE queue: the single ctx
                    # tile must free before the next step's PV matmuls
                    if c == C - 1:
                        for p in range(NPAIR):
                            nc.vector.tensor_copy(stage[:, p],
                                                  cps[p][:, 0:2, :])
                    else:
                        nc.vector.tensor_copy(stage[:], cps[:, :, 0:2, :])
                    nc.sync.dma_start(out_d[c], stage[:])
                if j is not None:
                    # masks deferred to step end: PV only reads the masked
                    # slices one step later, and this keeps the DVE queue
                    # free for the evac + proj evacuations mid-step
                    for p in range(NPAIR):
                        h0 = p * 2
                        if j == 0:
                            nc.vector.tensor_mul(
                                pj[:, h0:h0 + 2, 2, :], pj[:, h0:h0 + 2, 2, :],
                                masks[:, :, 1, :]
                            )
                        elif j == C - 1:
                            nc.vector.tensor_mul(
                                pj[:, h0:h0 + 2, 0, :], pj[:, h0:h0 + 2, 0, :],
                                masks[:, :, 0, :]
                            )
                        else:
                            nc.vector.tensor_mul(
                                pj[:, h0:h0 + 2, 0:3:2, :],
                                pj[:, h0:h0 + 2, 0:3:2, :],
                                masks[:]
                            )
                for u in fillers[1:]:
                    u()

            # stripe-n projections run one group ahead of the attention steps
            # they unblock; pending proj units are spread between j-steps as
            # TensorE filler while ScalarE digests the exps.
            for p in range(NPAIR):
                emit_qk_proj_unit(0, p)
            s0_v = [lambda: emit_v_proj_unit(0), lambda: emit_v_proj_unit(2)]
            for n in range(NSTRIPE):
                if n == 0:
                    js = list(range(0, 3))
                elif n < NSTRIPE - 1:
                    js = list(range(4 * n - 1, 4 * n + 3))
                else:
                    js = list(range(4 * n - 1, C))
                pending = proj_units(n + 1) if n + 1 < NSTRIPE else []
                if n == 0:
                    pending = s0_v + pending
                total = len(pending)
                taken = 0
                for i, j in enumerate(js):
                    want = -((-total * (i + 1)) // len(js))  # front-loaded
                    fillers = []
                    while taken < want:
                        fillers.append(pending.pop(0))
                        taken += 1
                    emit_step(j, j - 2 if j >= 2 else None, fillers)
            emit_step(None, C - 2)
            emit_step(None, C - 1)

    nc.compile()
    return nc


def _get_nc():
    if "nc" not in _CACHE:
        _CACHE["nc"] = _build_nc()
    return _CACHE["nc"]


def kernel(hidden_states, Wq, bq, Wk, bk, Wv, bv):
    from concourse.bass_utils import run_bass_kernel_spmd
    import os

    nc = _get_nc()

    hidden_states = np.asarray(hidden_states, np.float32)
    Wq, Wk, Wv = (np.asarray(w, np.float32) for w in (Wq, Wk, Wv))
    bv = np.asarray(bv, np.float32)

    # triangular band masks (bf16 0/1), packed [128, (headdup 2, slice 2, 128)]
    y = np.arange(128)[:, None]
    x = np.arange(128)[None, :]
    m0 = (x >= y).astype(np.float32)   # slice 0: chunk j-1
    m2 = (x <= y).astype(np.float32)   # slice 2: chunk j+1
    mp = np.stack([m0, m2], axis=1)                  # [128, 2, 128]
    masks = np.broadcast_to(mp[:, None], (128, 2, 2, 128))
    masks = np.ascontiguousarray(masks).reshape(128, 512).astype(BF16)

    SBH = KD * 512
    wslice = {}
    for g in range(2):
        sl = slice(g * NHL * D, (g + 1) * NHL * D)
        def pack(w):
            # [768, 384] W.T -> pair-major SBUF layout [128, NPAIR*KD*128]
            wt = np.ascontiguousarray(w[sl, :].T)
            return np.ascontiguousarray(
                wt.reshape(KD, 128, NPAIR, 128).transpose(1, 2, 0, 3)
                .reshape(128, NPAIR * KD * 128)).astype(BF16)
        wslice[g] = (pack(Wq), pack(Wk), pack(Wv))

    in_maps = []
    for b in range(B):
        # stripe-major: [128, NSTRIPE, KD, 512] flattened
        hTb = np.ascontiguousarray(hidden_states[b].T)  # [768, 4096]
        hT = np.ascontiguousarray(
            hTb.reshape(KD, 128, S // 512, 512).transpose(1, 2, 0, 3)
            .reshape(128, HID * S // 128)).astype(BF16)
        for g in range(2):
            wqg, wkg, wvg = wslice[g]
            in_maps.append(
                {"hT": hT, "wv": wvg, "masks": masks,
                 "bun1": np.concatenate([wqg, hT[:, 0:SBH // 2]], axis=1),
                 "bun2": np.concatenate(
                     [wkg, hT[:, SBH // 2:SBH]], axis=1)}
            )

    trace = bool(int(os.environ.get("KERNEL_TRACE", "0")))
    res = run_bass_kernel_spmd(nc, in_maps, list(range(8)), trace=trace)
    _CACHE["last_result"] = res

    out = np.empty((B, S, HID), np.float32)
    for b in range(B):
        for g in range(2):
            o = res.results[b * 2 + g]["out"].astype(np.float32)
            # [C, 65, 3, 2, 128] -> heads h = p*2+sub
            o = o.reshape(C, D + 1, NHL, W)
            ctx = o[:, :D] / o[:, D:D + 1]          # [C, 64, 6, 128]
            ctx = ctx.transpose(0, 3, 2, 1).reshape(S, NHL, D)
            ctx = ctx + bv[g * NHL * D:(g + 1) * NHL * D].reshape(1, NHL, D)
            out[b, :, g * NHL * D:(g + 1) * NHL * D] = ctx.reshape(S, NHL * D)
    return out


# revision 3
# speedup vs baseline: 1.0034x; 1.0034x over previous
# Banded (sliding-window) attention kernel for 8 TRN2 NeuronCores — v2.
#
# Problem: B=4, S=4096, HID=768, NH=12, D=64, one-sided window W=128.
# Sharding: core = b*2 + g (4 batches x 2 head-groups of 6 heads), fully
# independent cores, no collectives.
#
# v2 changes over v1 (213us):
#  - hT lives stripe-major ([128, stripe, ktile, 512]) so each projection
#    stripe arrives as ONE contiguous-6KB-line DMA.  Each DMA instruction
#    costs ~4.7us of queue service regardless of size, so ALL ramp-
#    critical inputs travel as exactly TWO bundle DMAs (sync: wq|wv|hT
#    stripe 0; scalar: wk|masks|hT stripe 1), host-concatenated; stripes
#    2-7 follow individually, alternating queues.
#    (fp8 was tried twice: full-fp8 DoubleRow projections are 2x faster
#    on PE but cost 2.1-3.7% norm rel vs the 2e-2 gate; a 1/3-fp8 hybrid
#    DR matmul appended to the bf16 chains measured 1.1e-2 error but ran
#    27us SLOWER -- the DR matmul stalls the bf16 LDW/MM pipeline.)
#  - Score matmuls for a head pair are emitted back-to-back with explicit
#    tile_position (0,0)/(64,0) so the two K=64 matmuls run concurrently
#    in separate 64-row groups of the PE array.
#  - ctx PSUM for all 6 heads of a chunk lives in one 3-bank tile
#    [65, 3pair, 4(sub0,sub1,pad,pad), 128]; one strided DVE copy
#    evacuates it (925ns vs 3x400) and one DMA per chunk ships it.
#  - Output is bf16 (half the output DMA bytes).
#
# Per-core pipeline (bf16 TensorE math, f32 PSUM accumulation):
#   for each 512-col projection stripe: project Q,K (d-major per pair)
#   and V (s-major, ones column for the softmax denominator), then run
#   the banded-attention key-tiles the stripe unblocks:
#     key-tile j: scores S_T[y, x] = K_j^T Q over query span (j-1..j+2)*128
#     (per-head PSUM bank), exp on ScalarE (band scale fused), triangular
#     0/1 band masks on VectorE, then PV with V stationary accumulating
#     ctx_T [65, 3, 4, 128] per chunk, one DVE evacuation, one DMA.
#   Normalization (divide by denominator) and the V-bias add happen on host.
#
# Output per core: [C=32, 65, 3, 2, 128] bf16 = (chunk, d|rowsum, pair, sub, x).

import numpy as np
import ml_dtypes

B, S, HID, NH, W = 4, 4096, 768, 12, 128
D = HID // NH          # 64
C = S // W             # 32 chunks / key-tiles
NHL = 6                # heads per core
NPAIR = 3              # head pairs per core (2 heads share 128 partitions)
KD = HID // 128        # 6 hidden k-tiles
BF16 = ml_dtypes.bfloat16

_CACHE = {}


def _build_nc():
    import concourse.bacc as bacc
    import concourse.tile as tile
    from concourse import mybir

    f32 = mybir.dt.float32
    bf16 = mybir.dt.bfloat16

    nc = bacc.Bacc(
        "TRN2", target_bir_lowering=False, debug=False, num_devices=8
    )

    # hT host-packed stripe-major [128, NSTRIPE, KD, 512]; weights
    # host-packed pair-major [128, NPAIR, KD, 128].
    WCOLS = NPAIR * KD * 128   # 2304 weight cols per tensor
    SBH = KD * 512             # 3072 cols per hT stripe
    hT_d = nc.dram_tensor("hT", [128, HID * S // 128], bf16, kind="ExternalInput")
    # ramp bundles: each fast queue gets ONE ~960KB DMA carrying a weight
    # tensor + half of stripe 0 (per-DMA queue service is ~4.7us + ~bytes/
    # 130GB/s, so fewer medium DMAs beat both many-small and one-huge)
    bun1_d = nc.dram_tensor("bun1", [128, WCOLS + SBH // 2], bf16,
                            kind="ExternalInput")
    bun2_d = nc.dram_tensor("bun2", [128, WCOLS + SBH // 2], bf16,
                            kind="ExternalInput")
    wv_d = nc.dram_tensor("wv", [128, WCOLS], bf16, kind="ExternalInput")
    mask_d = nc.dram_tensor("masks", [128, 4 * W], bf16, kind="ExternalInput")
    out_d = nc.dram_tensor("out", [C, D + 1, NPAIR, 2, W], bf16,
                           kind="ExternalOutput")

    NS = 512               # projection stripe (free dim)
    NSTRIPE = S // NS      # 8

    with tile.TileContext(nc) as tc:
        with (
            tc.tile_pool(name="persist", bufs=1) as persist,
            tc.tile_pool(name="probs", bufs=4) as probs_pool,
            tc.tile_pool(name="stage", bufs=3) as stage_pool,
            # PSUM: 8 banks. proj 2x1, score 3x1, ctx 1x3. Consecutive
            # matmuls must hit different banks (same-bank back-to-back
            # serializes on the ~166ns pipeline drain), so independent
            # chains are interleaved everywhere below.
            tc.tile_pool(name="proj_ps", bufs=2, space="PSUM") as proj_ps,
            tc.tile_pool(name="score_ps", bufs=3, space="PSUM") as score_ps,
            tc.tile_pool(name="ctx_ps", bufs=1, space="PSUM") as ctx_ps,
        ):
            # ---- persistent SBUF buffers ----
            hT = persist.tile([128, NSTRIPE, KD, NS], bf16, tag="hT")
            bun1 = persist.tile([128, WCOLS + SBH // 2], bf16, tag="bun1")
            bun2 = persist.tile([128, WCOLS + SBH // 2], bf16, tag="bun2")
            wv = persist.tile([128, NPAIR, KD, 128], bf16, tag="wv")
            masks = persist.tile([128, 2, 2, W], bf16, tag="masks")

            def wview(bun):
                return bun[:, 0:WCOLS].rearrange(
                    "p (a k c) -> p a k c", a=NPAIR, k=KD, c=128)

            wq = wview(bun1)
            wk = wview(bun2)
            hs0a = bun1[:, WCOLS:].rearrange("p (k c) -> p k c", k=3, c=NS)
            hs0b = bun2[:, WCOLS:].rearrange("p (k c) -> p k c", k=3, c=NS)

            def hTs(n, k):
                # [128, 512] slice of stripe n, k-tile k
                if n == 0:
                    return (hs0a, hs0b)[k // 3][:, k % 3, :]
                return hT[:, n, k, :]
            qdm = [persist.tile([128, S], bf16, tag=f"q{p}", name=f"q{p}")
                   for p in range(NPAIR)]
            kdm = [persist.tile([128, S], bf16, tag=f"k{p}", name=f"k{p}")
                   for p in range(NPAIR)]
            # V s-major with interleaved denominator column: [s-tile, head, 65]
            vsm = persist.tile([128, C, NHL, D + 1], bf16, tag="vsm")

            # ---- input DMAs.  Queue completion ~= 4.7us latency + bytes
            # at ~130GB/s, FIFO per queue: order strictly by needed-by time,
            # smallest-first, balanced across the two fast queues.
            PBW = KD * 128  # bundle cols per weight pair
            nc.sync.dma_start(bun1[:, 0:PBW], bun1_d[:, 0:PBW])
            nc.scalar.dma_start(bun2[:, 0:PBW], bun2_d[:, 0:PBW])
            nc.sync.dma_start(bun1[:, WCOLS:], bun1_d[:, WCOLS:])
            nc.scalar.dma_start(bun2[:, WCOLS:], bun2_d[:, WCOLS:])
            nc.sync.dma_start(bun1[:, PBW:WCOLS], bun1_d[:, PBW:WCOLS])
            nc.scalar.dma_start(bun2[:, PBW:WCOLS], bun2_d[:, PBW:WCOLS])
            nc.gpsimd.dma_start(wv[:], wv_d[:])
            nc.gpsimd.dma_start(masks[:], mask_d[:])
            HB = SBH // 2
            nc.sync.dma_start(hT[:, 1, 0:3], hT_d[:, SBH:SBH + HB])
            nc.scalar.dma_start(hT[:, 1, 3:6], hT_d[:, SBH + HB:2 * SBH])
            for n in range(2, NSTRIPE):
                eng = (nc.scalar, nc.sync)[n % 2]
                eng.dma_start(hT[:, n], hT_d[:, n * SBH:(n + 1) * SBH])
            # ones column for the PV denominator
            nc.vector.memset(vsm[:, :, :, D:D + 1], 1.0)

            # ---- HAM warm-up: ~6us of zero matmuls while the ramp DMAs
            # land, so the PE clock-gate is at 8/8 when real work starts.
            warm_sb = persist.tile([128, NS], bf16, tag="warm")
            # nonzero, varying data: an all-zeros matmul may not register
            # as activity on the PE clock-gate's monitor
            nc.gpsimd.iota(warm_sb[:], [[1, NS]], base=1, channel_multiplier=3,
                           allow_small_or_imprecise_dtypes=True)
            wps = [proj_ps.tile([128, NS], f32, tag="proj", name="warm_ps")
                   for _ in range(2)]
            for i in range(14):
                nc.tensor.matmul(
                    wps[i % 2][:], warm_sb[:, 0:128], warm_sb[:],
                    start=True, stop=True,
                )

            # ---- fused projection + attention pipeline ----
            ptiles = [None] * C

            def emit_qk_proj_unit(n, p):
                # Q and K accumulation chains interleaved (alternating banks)
                psq = proj_ps.tile([128, NS], f32, tag="proj",
                                   name="proj_ps_q")
                psk = proj_ps.tile([128, NS], f32, tag="proj",
                                   name="proj_ps_k")
                for k in range(KD):
                    for ps, w in ((psq, wq), (psk, wk)):
                        nc.tensor.matmul(
                            ps[:],
                            w[:, p, k, :],
                            hTs(n, k),
                            start=(k == 0), stop=(k == KD - 1),
                        )
                nc.vector.tensor_copy(qdm[p][:, n * NS:(n + 1) * NS], psq[:])
                nc.vector.tensor_copy(kdm[p][:, n * NS:(n + 1) * NS], psk[:])

            def emit_v_proj_unit(sta):
                # two V s-tile chains interleaved
                psa = proj_ps.tile([128, NHL, D], f32, tag="proj",
                                   name="vproj_ps_a")
                psb = proj_ps.tile([128, NHL, D], f32, tag="proj",
                                   name="vproj_ps_b")
                for k in range(KD):
                    for ps, st in ((psa, sta), (psb, sta + 1)):
                        nc.tensor.matmul(
                            ps[:],
                            hTs(st // 4, k)[:, (st % 4) * 128:(st % 4 + 1) * 128],
                            wv[:, :, k, :],
                            start=(k == 0), stop=(k == KD - 1),
                        )
                nc.vector.tensor_copy(vsm[:, sta, :, 0:D], psa[:])
                nc.vector.tensor_copy(vsm[:, sta + 1, :, 0:D], psb[:])

            def proj_units(n, v_first=True):
                # As fillers, V units go first/early: their DVE evacuations
                # feed the next group's PV LDWEIGHTS.  In the prologue, ALL
                # QK units go first: pair-0 weights + stripe 0 are the only
                # ramp-critical DMAs; wv arrives while the QK chains run
                # (the PE queue is strict FIFO, so a stalled V unit would
                # block later QK units emitted behind it).
                qk = [lambda p=p: emit_qk_proj_unit(n, p) for p in range(NPAIR)]
                v = [lambda sta=sta: emit_v_proj_unit(sta)
                     for sta in (n * 4, n * 4 + 2)]
                if v_first:
                    return [v[0], qk[0], v[1], qk[1], qk[2]]
                return [qk[0], qk[1], qk[2], v[0], v[1]]

            def emit_step(j, c, fillers=()):
                # key-tile j scores (QK + exp + mask), interleaved with the
                # PV matmuls of chunk c = j-2.  P tile slice pi:
                # 0 -> chunk j-1 (mask x>=y), 1 -> chunk j, 2 -> chunk j+1
                # (mask x<=y).
                pv_mms = []
                if c is not None:
                    stage = stage_pool.tile([D + 1, NPAIR, 2, W],
                                            mybir.dt.bfloat16,
                                            tag="stage", name="stage_t")
                    ts = [t for t in (c - 1, c, c + 1) if 0 <= t < C]
                    if c == C - 1:
                        # the last chunk borrows 3 banks from the (by now
                        # idle) proj/score pools so its PV doesn't wait on
                        # the previous chunk's ctx evacuation
                        cps = [
                            proj_ps.tile([D + 1, 4, W], mybir.dt.float32,
                                         tag="proj", name="ctx_last_a"),
                            proj_ps.tile([D + 1, 4, W], mybir.dt.float32,
                                         tag="proj", name="ctx_last_b"),
                            score_ps.tile([D + 1, 4, W], mybir.dt.float32,
                                          tag="score", name="ctx_last_c"),
                        ]
                        sl = lambda p, sub: cps[p][:, sub, :]
                    else:
                        # one 3-bank ctx tile: [65, pair, sub(pad 4), 128]
                        cps = ctx_ps.tile([D + 1, NPAIR, 4, W],
                                          mybir.dt.float32,
                                          tag="ctx", name="ctx_ps_t")
                        sl = lambda p, sub: cps[:, p, sub, :]
                    # sub-outer: groups sharing a ctx bank stay sequential
                    # (start=True clears the whole bank's has_written bits);
                    # pair-inner: consecutive matmuls rotate across the 3
                    # ctx banks so they stream without drain serialization.
                    for sub in range(2):
                        for i, t in enumerate(ts):
                            for p in range(NPAIR):
                                pv_mms.append((
                                    sl(p, sub),
                                    vsm[:, t, p * 2 + sub, :],
                                    (t, p * 2 + sub, c - t + 1),
                                    i == 0, i == len(ts) - 1,
                                ))

                def drain_pv(k):
                    # no PV before h2: chunk c's ctx tile is WAR-blocked on
                    # chunk c-1's DVE evacuation (ctx pool rotation distance
                    # is one chunk); by h2 that has long retired.
                    if k < 2:
                        return
                    while pv_mms:
                        out, lhsT, (t, h, pi), st_, sp_ = pv_mms.pop(0)
                        nc.tensor.matmul(
                            out, lhsT, ptiles[t][:, h, pi, :],
                            start=st_, stop=sp_,
                        )

                if j is not None:
                    x0 = max(0, (j - 1) * 128)
                    x1 = min(S, (j + 2) * 128)
                    c0 = x0 - (j - 1) * 128
                    c1 = c0 + (x1 - x0)
                    s0, s1 = c0 // 128, (c1 - 1) // 128 + 1
                    pj = probs_pool.tile([128, NHL, 3, W], bf16, tag="P",
                                         name="P_t")
                    ptiles[j] = pj
                    for p in range(NPAIR):
                        if p == 2 and fillers:
                            # a ~1.3us projection chain absorbs the
                            # score-bank WAR wait (pair 2's banks free only
                            # after earlier exps retire)
                            fillers[0]()
                        pss = []
                        # the two K=64 score matmuls of a pair target
                        # disjoint 64-row groups -> back-to-back emission
                        # lets them run concurrently in the PE array
                        for sub in range(2):
                            bp = sub * 64
                            ps = score_ps.tile([128, 4 * W], f32, tag="score",
                                               name="score_ps_t")
                            pss.append(ps)
                            nc.tensor.matmul(
                                ps[:, c0:c1],
                                kdm[p][bp:bp + 64, j * 128:(j + 1) * 128],
                                qdm[p][bp:bp + 64, x0:x1],
                                start=True, stop=True,
                                tile_position=(bp, 0),
                            )
                        for sub in range(2):
                            h = p * 2 + sub
                            nc.scalar.activation(
                                pj[:, h, s0:s1, :], pss[sub][:, c0:c1],
                                mybir.ActivationFunctionType.Exp,
                                scale=1.0 / float(np.sqrt(D)),
                            )
                            drain_pv(p * 2 + sub)
                drain_pv(5)
                if c is not None:
                    # ctx evacuation FIRST on the DVE queue: the single ctx
                    # tile must free before the next step's PV matmuls
                    if c == C - 1:
                        for p in range(NPAIR):
                            nc.vector.tensor_copy(stage[:, p],
                                                  cps[p][:, 0:2, :])
                    else:
                        nc.vector.tensor_copy(stage[:], cps[:, :, 0:2, :])
                    nc.sync.dma_start(out_d[c], stage[:])
                if j is not None:
                    # masks deferred to step end: PV only reads the masked
                    # slices one step later, and this keeps the DVE queue
                    # free for the evac + proj evacuations mid-step
                    for p in range(NPAIR):
                        h0 = p * 2
                        if j == 0:
                            nc.vector.tensor_mul(
                                pj[:, h0:h0 + 2, 2, :], pj[:, h0:h0 + 2, 2, :],
                                masks[:, :, 1, :]
                            )
                        elif j == C - 1:
                            nc.vector.tensor_mul(
                                pj[:, h0:h0 + 2, 0, :], pj[:, h0:h0 + 2, 0, :],
                                masks[:, :, 0, :]
                            )
                        else:
                            nc.vector.tensor_mul(
                                pj[:, h0:h0 + 2, 0:3:2, :],
                                pj[:, h0:h0 + 2, 0:3:2, :],
                                masks[:]
                            )
                for u in fillers[1:]:
                    u()

            # stripe-n projections run one group ahead of the attention steps
            # they unblock; pending proj units are spread between j-steps as
            # TensorE filler while ScalarE digests the exps.
            for p in range(NPAIR):
                emit_qk_proj_unit(0, p)
            s0_v = [lambda: emit_v_proj_unit(0), lambda: emit_v_proj_unit(2)]
            for n in range(NSTRIPE):
                if n == 0:
                    js = list(range(0, 3))
                elif n < NSTRIPE - 1:
                    js = list(range(4 * n - 1, 4 * n + 3))
                else:
                    js = list(range(4 * n - 1, C))
                pending = proj_units(n + 1) if n + 1 < NSTRIPE else []
                if n == 0:
                    pending = s0_v + pending
                total = len(pending)
                taken = 0
                for i, j in enumerate(js):
                    want = -((-total * (i + 1)) // len(js))  # front-loaded
                    fillers = []
                    while taken < want:
                        fillers.append(pending.pop(0))
                        taken += 1
                    emit_step(j, j - 2 if j >= 2 else None, fillers)
            emit_step(None, C - 2)
            emit_step(None, C - 1)

    nc.compile()
    return nc


def _get_nc():
    if "nc" not in _CACHE:
        _CACHE["nc"] = _build_nc()
    return _CACHE["nc"]


def kernel(hidden_states, Wq, bq, Wk, bk, Wv, bv):
    from concourse.bass_utils import run_bass_kernel_spmd
    import os

    nc = _get_nc()

    hidden_states = np.asarray(hidden_states, np.float32)
    Wq, Wk, Wv = (np.asarray(w, np.float32) for w in (Wq, Wk, Wv))
    bv = np.asarray(bv, np.float32)

    # triangular band masks (bf16 0/1), packed [128, (headdup 2, slice 2, 128)]
    y = np.arange(128)[:, None]
    x = np.arange(128)[None, :]
    m0 = (x >= y).astype(np.float32)   # slice 0: chunk j-1
    m2 = (x <= y).astype(np.float32)   # slice 2: chunk j+1
    mp = np.stack([m0, m2], axis=1)                  # [128, 2, 128]
    masks = np.broadcast_to(mp[:, None], (128, 2, 2, 128))
    masks = np.ascontiguousarray(masks).reshape(128, 512).astype(BF16)

    SBH = KD * 512
    wslice = {}
    for g in range(2):
        sl = slice(g * NHL * D, (g + 1) * NHL * D)
        def pack(w):
            # [768, 384] W.T -> pair-major SBUF layout [128, NPAIR*KD*128]
            wt = np.ascontiguousarray(w[sl, :].T)
            return np.ascontiguousarray(
                wt.reshape(KD, 128, NPAIR, 128).transpose(1, 2, 0, 3)
                .reshape(128, NPAIR * KD * 128)).astype(BF16)
        wslice[g] = (pack(Wq), pack(Wk), pack(Wv))

    in_maps = []
    for b in range(B):
        # stripe-major: [128, NSTRIPE, KD, 512] flattened
        hTb = np.ascontiguousarray(hidden_states[b].T)  # [768, 4096]
        hT = np.ascontiguousarray(
            hTb.reshape(KD, 128, S // 512, 512).transpose(1, 2, 0, 3)
            .reshape(128, HID * S // 128)).astype(BF16)
        for g in range(2):
            wqg, wkg, wvg = wslice[g]
            in_maps.append(
                {"hT": hT, "wv": wvg, "masks": masks,
                 "bun1": np.concatenate([wqg, hT[:, 0:SBH // 2]], axis=1),
                 "bun2": np.concatenate(
                     [wkg, hT[:, SBH // 2:SBH]], axis=1)}
            )

    trace = bool(int(os.environ.get("KERNEL_TRACE", "0")))
    res = run_bass_kernel_spmd(nc, in_maps, list(range(8)), trace=trace)
    _CACHE["last_result"] = res

    out = np.empty((B, S, HID), np.float32)
    for b in range(B):
        for g in range(2):
            o = res.results[b * 2 + g]["out"].astype(np.float32)
            # [C, 65, 3, 2, 128] -> heads h = p*2+sub
            o = o.reshape(C, D + 1, NHL, W)
            ctx = o[:, :D] / o[:, D:D + 1]          # [C, 64, 6, 128]
            ctx = ctx.transpose(0, 3, 2, 1).reshape(S, NHL, D)
            ctx = ctx + bv[g * NHL * D:(g + 1) * NHL * D].reshape(1, NHL, D)
            out[b, :, g * NHL * D:(g + 1) * NHL * D] = ctx.reshape(S, NHL * D)
    return out
